# revision 72
# baseline (speedup 1.0000x reference)
"""Multi-relation GAT layer on 8 Trainium2 NeuronCores.

Sharding: cores 0-3 own batch 0, cores 4-7 own batch 1; within a batch each
core owns a quarter of the destination rows (IS=512) for ALL R relations, so
the relation-mean, residual and LayerNorm stay core-local (no collectives).

Host precomputes the dense projections (Wh = H@W, attention dots es/ed) and
all scalar exponentials; the device does the heavy O(R*Hh*N*IS) masked
attention.

Algebra: scores are rank-1 before the leaky-relu (s[i,j] = es_i + ed_j);
softmax over j is invariant to per-i scaling, and per-j scaling folds into
the aggregation weights (linear over j). Dividing exp(lrelu(s)) by exp(es_i)
(per-i) and t_j = exp(0.2*ed_j) (per-j, folded into Wh) leaves

    u[j,i] = m[j,i] * max(invtau_j, g_i)
      invtau = exp(0.8*ed), g = exp(-0.8*es)   (host-precomputed; rescaled
                                               per (r,b,h) so invtau <= 1 --
                                               alpha is scale-invariant)
      W~h = Wh * t,  ones column = R * t       (host-folded)

The max is decomposed as invtau_j + relu(g_i - invtau_j), which splits the
aggregation into a mask-only matmul plus a correction:

    agg^T = (W~h * invtau)^T @ m  +  W~h^T @ (m * relu(g - invtau))

Term 1 (lhsT host-folded, incl. the R*t*invtau denominator column) depends
only on the mask, so the PE runs it while the elementwise engines produce
the correction. The relu units (one per (jt, h), per-partition scalar
invtau) are load-balanced between ACT (Relu activation, bias = -invtau) and
DVE (tensor_scalar sub+max, 2x mode); the mask multiply is a single batched
2x-mode tensor_tensor per j-tile group with the mask broadcast over heads
via a stride-0 middle AP dim.

PSUM: two banks per relation (double-buffered), two heads per bank at
32-aligned partition bases 0/64, so per-head matmul pairs land in distinct
PE column groups and run concurrently (~30ns stagger). Term-1 jt0 carries
start=True; term-2 jt15 carries stop.

Epilogue per (r, i-tile): whole-bank ACT drain to SBUF, one paired PE
transpose per bank to [i, (f,den)] layout, ACT gathers the four denominator
columns, DVE reciprocal, then fused DVE STTs do acc_h = num_h*rec_h +
(residual | acc_h). Pipelined one relation behind the hot loop. Final LN:
negated mean via STT accumulation, variance on ACT (Square, bias=-mu,
accumulated), Sqrt table preloaded at startup.

(GpSimd/Pool is left idle on purpose: tensor ops there measured ~30x slow
and stall the DVE via shared SBUF ports.)
"""

import sys

sys.path.insert(0, "/opt/trn_rl_repo")

import numpy as np

R, B, N, D, Hh, hd = 3, 2, 2048, 128, 4, 32
NCORES = 8
NQ = 4  # i-shards per batch
IS = N // NQ  # 512 dst rows per core
NIT = IS // 128  # 4 partition tiles of dst rows
NT = N // 128  # 16 j tiles
LN_EPS = 1e-5
HW = Hh * 33  # 132 packed Wh cols per j-tile (32 wh + 1 ones per head)
GJT = 4  # max j-tiles per v/u buffer
GROUPS = [(0, 4), (4, 4), (8, 4), (12, 3), (15, 1)]  # (start jt, len) taper
NDH = 4  # all heads use the max = invtau + relu(g - invtau) decomposition
W1 = NDH * 33  # term-1 (mask-matmul) lhsT columns: (wh*invtau | R*t*invtau)
ACT_PER_R = 36  # of NT*Hh=64 relu units per relation: how many on ACT ...
GPS_PER_R = 0  # ... and on GpSimd (tensor ops measured ~30x slow there!)

_CACHE = {}


def _build_program(trivial_ln=False):
    import concourse.bass as bass
    import concourse.mybir as mybir
    import concourse.tile as tile
    from concourse import bacc
    from concourse.masks import make_identity
    from contextlib import ExitStack

    f32 = mybir.dt.float32
    f16 = mybir.dt.float16
    Alu = mybir.AluOpType
    Act = mybir.ActivationFunctionType

    nc = bacc.Bacc("TRN2", target_bir_lowering=False, debug=False)
    mq = nc.declare_dram_parameter("mq", [R, 128, NT * IS], f16, isOutput=False)
    whp = nc.declare_dram_parameter("whp", [R, 128, NT * HW], f16, isOutput=False)
    whp1 = nc.declare_dram_parameter("whp1", [R, 128, NT * W1], f16, isOutput=False)
    gb4 = nc.declare_dram_parameter("gb4", [R, 128, Hh * IS], f16, isOutput=False)
    itc = nc.declare_dram_parameter("itc", [R, 128, NT * Hh], f32, isOutput=False)
    itn = nc.declare_dram_parameter("itn", [R, 128, NT * Hh], f32, isOutput=False)
    hres = nc.declare_dram_parameter("hres", [NIT, 128, D], f32, isOutput=False)
    gmb = nc.declare_dram_parameter("gmb", [2, 128, D], f32, isOutput=False)
    out = nc.declare_dram_parameter("out", [NIT, 128, D], f32, isOutput=True)

    with ExitStack() as ctx:
        tc = ctx.enter_context(tile.TileContext(nc))
        const = ctx.enter_context(tc.tile_pool(name="const", bufs=1))
        mq_pool = ctx.enter_context(tc.tile_pool(name="mq", bufs=2))
        mq0_pool = ctx.enter_context(tc.tile_pool(name="mq0", bufs=3))
        v_pool = ctx.enter_context(tc.tile_pool(name="v", bufs=2))
        u_pool = ctx.enter_context(tc.tile_pool(name="u", bufs=3))
        aggsb_pool = ctx.enter_context(tc.tile_pool(name="aggsb", bufs=3))
        small = ctx.enter_context(tc.tile_pool(name="small", bufs=8))
        epi_pool = ctx.enter_context(tc.tile_pool(name="epi", bufs=6))
        psum_agg = ctx.enter_context(tc.tile_pool(name="pagg", bufs=2, space="PSUM"))
        psum_tp = ctx.enter_context(tc.tile_pool(name="ptp", bufs=4, space="PSUM"))

        # ---- constants / per-relation operands ----
        # DMA priority order: r=0's itc + the h=0 slice of gb4 first (unblocks
        # the first v-units), then the first mask group (unblocks the first
        # TT), then whp[0] (unblocks matmuls), then everything else.
        ident = const.tile([128, 128], f32, tag="ident")

        whp_sb, whp1_sb, gb4_sb, itc_sb, itn_sb = [], [], [], [], []
        mq_pre = {}
        for r in range(R):
            q = const.tile([128, NT * Hh], f32, tag=f"itc{r}")
            itc_sb.append(q)
            q2 = const.tile([128, NT * Hh], f32, tag=f"itn{r}")
            itn_sb.append(q2)
            g = const.tile([128, Hh * IS], f16, tag=f"gb4{r}")
            gb4_sb.append(g)
            w = const.tile([128, NT * HW], f16, tag=f"whp{r}")
            whp_sb.append(w)
            w1 = const.tile([128, NT * W1], f16, tag=f"whp1_{r}")
            whp1_sb.append(w1)

        nc.sync.dma_start(itc_sb[0][:], itc[0])
        nc.sync.dma_start(gb4_sb[0][:], gb4[0])
        # itn = -itc computed on-device: saves early DMA descriptor issues
        nc.vector.tensor_scalar_mul(itn_sb[0][:], itc_sb[0][:], -1.0)
        tiles = []
        for k, (j0, gsz) in enumerate(GROUPS):
            pool_k = mq0_pool if k == 0 else mq_pool
            m = pool_k.tile([128, gsz * IS], f16, tag=f"mqg{k}", name=f"mq0_{k}")
            nc.sync.dma_start(m[:], mq[0][:, j0 * IS : (j0 + gsz) * IS])
            tiles.append(m)
            if k == 0:  # whp right after the lead mask
                nc.sync.dma_start(whp1_sb[0][:], whp1[0])
                nc.sync.dma_start(whp_sb[0][:], whp[0])
        mq_pre[0] = tiles

        make_identity(nc, ident[:])

        hres_sb, acc = [], []
        for t in range(NIT):
            hh = const.tile([128, D], f32, tag=f"hres{t}")
            nc.sync.dma_start(hh[:], hres[t])
            hres_sb.append(hh)
            acc_t = const.tile([128, D], f32, tag=f"acc{t}", name=f"acc{t}")
            acc.append(acc_t)
        gam = const.tile([128, D], f32, tag="gam")
        nc.sync.dma_start(gam[:], gmb[0])
        bet = const.tile([128, D], f32, tag="bet")
        nc.sync.dma_start(bet[:], gmb[1])
        eps_b = const.tile([128, 1], f32, tag="eps_b")
        nc.gpsimd.memset(eps_b[:], LN_EPS)
        # Touch Sqrt once now so the ACT function-table load (1.3us) happens
        # during the DMA-bound prologue instead of inside the LN tail.
        warm = const.tile([128, 1], f32, tag="warm", name="warm")
        nc.scalar.activation(warm[:], eps_b[:], Act.Sqrt, bias=eps_b[:])

        # ---- hot loop over relations ----
        pend = []

        def _epi_units(item):
            er, asbs = item
            for it in range(NIT):
                def quad(it=it):
                    # stage-ordered to avoid DVE<->PE ping-pong: paired
                    # transposes (two heads per bank), dens gather (ACT),
                    # recip, then fused (num*rec)+acc STTs on the DVE
                    tps = []
                    for bk in range(2):
                        tp = psum_tp.tile([128, 97], f32, tag="tp", name="tp")
                        at, _ = asbs[bk * 2]
                        nc.tensor.transpose(
                            tp[:],
                            at[0:97, it * 128 : (it + 1) * 128],
                            ident[0:97, 0:97],
                        )
                        tps.append(tp)
                    dens = small.tile([128, Hh], f32, tag="dens", name="dens")
                    for h in range(Hh):
                        c0 = (h % 2) * 64
                        nc.scalar.copy(
                            dens[:, h : h + 1], tps[h // 2][:, c0 + 32 : c0 + 33]
                        )
                    recs = small.tile([128, Hh], f32, tag="recs", name="recs")
                    nc.vector.reciprocal(recs[:], dens[:])
                    for h in range(Hh):
                        c0 = (h % 2) * 64
                        dst = acc[it][:, h * hd : (h + 1) * hd]
                        # acc_h = num_h * rec_h + (residual | acc_h)
                        nc.vector.scalar_tensor_tensor(
                            out=dst,
                            in0=tps[h // 2][:, c0 : c0 + 32],
                            scalar=recs[:, h : h + 1],
                            in1=hres_sb[it][:, h * hd : (h + 1) * hd]
                            if er == 0
                            else dst,
                            op0=Alu.mult,
                            op1=Alu.add,
                        )
                yield quad

        def _emit_epi(item):
            for u_ in _epi_units(item):
                u_()

        # Engine schedule for the relu units. First group (jt<4) stays
        # DVE-heavy: at relation boundaries the ACT queue is still draining
        # the previous epilogue, and the group-0 TT would stall on it.
        NU = NT * Hh
        G0U = GROUPS[0][1] * Hh  # units in the first group
        act_g0 = 4
        act_rest = ACT_PER_R - act_g0
        pattern = []
        accb = 0.0
        for ui in range(NU):
            if ui < G0U:
                pattern.append("A" if ui % Hh == 1 and ui // Hh < act_g0 else "D")
            else:
                accb += act_rest / (NU - G0U)
                if accb >= 1.0:
                    accb -= 1.0
                    pattern.append("A")
                else:
                    pattern.append("D")

        for r in range(R):
            if r + 1 < R:
                # defer the next relation's constants: keeps the early DMA
                # window clear for relation 0's masks
                nc.sync.dma_start(itc_sb[r + 1][:], itc[r + 1])
                nc.vector.tensor_scalar_mul(itn_sb[r + 1][:], itc_sb[r + 1][:], -1.0)
                nc.sync.dma_start(gb4_sb[r + 1][:], gb4[r + 1])
                nc.sync.dma_start(whp1_sb[r + 1][:], whp1[r + 1])
                nc.sync.dma_start(whp_sb[r + 1][:], whp[r + 1])
            if r in mq_pre:
                m_tiles = mq_pre[r]
            else:
                m_tiles = []
                for k, (j0, gsz) in enumerate(GROUPS):
                    pool_k = mq0_pool if k == 0 else mq_pool
                    m = pool_k.tile(
                        [128, gsz * IS], f16, tag=f"mqg{k}", name=f"mq{r}_{k}"
                    )
                    nc.sync.dma_start(m[:], mq[r][:, j0 * IS : (j0 + gsz) * IS])
                    m_tiles.append(m)

            # Two PSUM banks, two heads each at 32-aligned bases (0 and 64):
            # pairs of matmuls hit distinct PE column groups and run
            # concurrently (~4ns stagger).
            aggA = psum_agg.tile([97, IS], f32, tag="aggA", name=f"aggA{r}")
            aggB = psum_agg.tile([97, IS], f32, tag="aggB", name=f"aggB{r}")
            aggp = [aggA[0:33, :], aggA[64:97, :], aggB[0:33, :], aggB[64:97, :]]
            epi_iter = iter(_epi_units(pend.pop(0))) if pend else None

            for g, (j0, gsz) in enumerate(GROUPS):
                # term-1: (whp*invtau)^T @ mask — depends only on the mask
                # DMA, so the PE can run these while DVE/ACT produce p
                for jl in range(gsz):
                    jt = j0 + jl
                    for h in range(Hh):
                        nc.tensor.matmul(
                            aggp[h],
                            lhsT=whp1_sb[r][:, jt * W1 + h * 33 : jt * W1 + (h + 1) * 33],
                            rhs=m_tiles[g][:, jl * IS : (jl + 1) * IS],
                            start=(jt == 0),
                            stop=False,
                        )
                v = v_pool.tile([128, GJT * Hh * IS], f16, tag="v")
                for jl in range(gsz):
                    jt = j0 + jl
                    for h in range(Hh):
                        vsl = v[:, (jl * Hh + h) * IS : (jl * Hh + h + 1) * IS]
                        gsl = gb4_sb[r][:, h * IS : (h + 1) * IS]
                        # p-pass input: relu(g_i - invtau_j)
                        eng = pattern[jt * Hh + h]
                        if eng == "A":
                            nc.scalar.activation(
                                vsl,
                                gsl,
                                Act.Relu,
                                bias=itn_sb[r][:, jt * Hh + h : jt * Hh + h + 1],
                            )
                        else:
                            veng = nc.vector if eng == "D" else nc.gpsimd
                            veng.tensor_scalar(
                                out=vsl,
                                in0=gsl,
                                scalar1=itc_sb[r][:, jt * Hh + h : jt * Hh + h + 1],
                                scalar2=0.0,
                                op0=Alu.subtract,
                                op1=Alu.max,
                            )
                u = u_pool.tile([128, GJT * Hh * IS], f16, tag="u")
                m3 = m_tiles[g][:].rearrange("p (a i) -> p a i", a=gsz)
                nc.vector.tensor_mul(
                    u[:, : gsz * Hh * IS].rearrange("p (a h i) -> p a h i", a=gsz, h=Hh),
                    v[:, : gsz * Hh * IS].rearrange("p (a h i) -> p a h i", a=gsz, h=Hh),
                    m3[:, :, None, :].broadcast_to([128, gsz, Hh, IS]),
                )
                for jl in range(gsz):
                    jt = j0 + jl
                    for h in range(Hh):
                        nc.tensor.matmul(
                            aggp[h],
                            lhsT=whp_sb[r][:, jt * HW + h * 33 : jt * HW + (h + 1) * 33],
                            rhs=u[:, (jl * Hh + h) * IS : (jl * Hh + h + 1) * IS],
                            start=False,
                            stop=(jt == NT - 1),
                        )
                if epi_iter is not None:
                    u_ = next(epi_iter, None)  # one it-quad per group
                    if u_ is not None:
                        u_()

            # ---- drain PSUM quickly (frees agg banks for next r's chains).
            # Whole-bank copies keep src/dst partitions aligned (elementwise
            # engines cannot shift partitions). Last relation: drain is on
            # the critical tail with the DVE idle, so split it ACT/DVE. ----
            asbA = aggsb_pool.tile([97, IS], f32, tag="aggsbA")
            asbB = aggsb_pool.tile([97, IS], f32, tag="aggsbB")
            nc.scalar.copy(asbA[:], aggA[:])
            if r == R - 1:
                nc.vector.tensor_copy(asbB[:], aggB[:])
            else:
                nc.scalar.copy(asbB[:], aggB[:])
            asbs = [(asbA, 0), (asbA, 64), (asbB, 0), (asbB, 64)]
            pend.append((r, asbs))

        # last relation: 4-wide batched per dst partition tile
        er, asbs = pend.pop(0)
        xs, mus, xcs, stds = [], [], [], []
        contribs = [
            epi_pool.tile([128, D], f32, tag="fincon", name=f"contribf{it}")
            for it in range(NIT)
        ]
        for it in range(NIT):
            tps = []
            for bk in range(2):
                tp = psum_tp.tile([128, 97], f32, tag="tp", name="tpf")
                at, _ = asbs[bk * 2]
                nc.tensor.transpose(
                    tp[:], at[0:97, it * 128 : (it + 1) * 128], ident[0:97, 0:97]
                )
                tps.append(tp)
            # all-DVE here: the DVE is idle in the tail while ACT round
            # trips would serialize the chain
            recsf = small.tile([128, Hh], f32, tag="recsf", name="recsf")
            for h in range(Hh):
                c0 = (h % 2) * 64
                nc.vector.reciprocal(
                    recsf[:, h : h + 1], tps[h // 2][:, c0 + 32 : c0 + 33]
                )
            for h in range(Hh):
                c0 = (h % 2) * 64
                nc.vector.tensor_scalar(
                    out=contribs[it][:, h * hd : (h + 1) * hd],
                    in0=tps[h // 2][:, c0 : c0 + 32],
                    scalar1=recsf[:, h : h + 1],
                    scalar2=None,
                    op0=Alu.mult,
                )
            # LN stages: x + (negated) mean on DVE; variance on ACT (Square
            # with bias=-mu, accumulated) so the two run concurrently
            x = epi_pool.tile([128, D], f32, tag="x", name=f"x{it}")
            musum = small.tile([128, 1], f32, tag="mu", name=f"mu{it}")
            nc.vector.scalar_tensor_tensor(
                out=x[:], in0=acc[it][:], scalar=0.0, in1=contribs[it][:],
                op0=Alu.add, op1=Alu.add, accum_out=musum[:],
            )
            nc.vector.tensor_scalar_mul(musum[:], musum[:], -1.0 / D)  # = -mu
            xc = epi_pool.tile([128, D], f32, tag="xc", name=f"xc{it}")
            nc.vector.tensor_scalar(
                out=xc[:], in0=x[:], scalar1=musum[:], scalar2=None,
                op0=Alu.add,
            )
            sq = epi_pool.tile([128, D], f32, tag="sq", name=f"sq{it}")
            vsum = small.tile([128, 1], f32, tag="vs", name=f"vs{it}")
            nc.scalar.activation(
                sq[:], x[:], Act.Square, bias=musum[:], accum_out=vsum[:]
            )
            std = small.tile([128, 1], f32, tag="std", name=f"std{it}")
            nc.scalar.activation(std[:], vsum[:], Act.Sqrt, bias=eps_b[:], scale=1.0 / D)
            xs.append(x)
            mus.append(musum)
            xcs.append(xc)
            stds.append(std)

        # ---- epilogue: LayerNorm finish (stage-batched) ----
        xcgs, rstds = [], []
        if not trivial_ln:
            for t in range(NIT):
                # fills the DVE while ACT does the sqrts (rstd-independent)
                xcg = epi_pool.tile([128, D], f32, tag="xg", name=f"xcg{t}")
                nc.vector.tensor_mul(xcg[:], xcs[t][:], gam[:])
                xcgs.append(xcg)
        for t in range(NIT):
            rstd = small.tile([128, 1], f32, tag="rstd", name=f"rstd{t}")
            nc.vector.reciprocal(rstd[:], stds[t][:])
            rstds.append(rstd)
        for t in range(NIT):
            xo = epi_pool.tile([128, D], f32, tag="xo", name=f"xo{t}")
            if trivial_ln:
                # gamma==1, beta==0 (checked at build time): xo = xc*rstd
                nc.vector.tensor_scalar(
                    out=xo[:], in0=xcs[t][:], scalar1=rstds[t][:],
                    scalar2=None, op0=Alu.mult,
                )
            else:
                # xo = (xc*gamma)*rstd + beta in one fused op
                nc.vector.scalar_tensor_tensor(
                    out=xo[:], in0=xcgs[t][:], scalar=rstds[t][:], in1=bet[:],
                    op0=Alu.mult, op1=Alu.add,
                )
            nc.sync.dma_start(out[t], xo[:])

    nc.compile()
    return nc


def _host_pack(H, A, W, a_src, a_dst, ln_gamma, ln_beta):
    H = np.asarray(H, np.float32)
    A = np.asarray(A)
    W = np.asarray(W, np.float32)
    a_src = np.asarray(a_src, np.float32)
    a_dst = np.asarray(a_dst, np.float32)
    ln_gamma = np.asarray(ln_gamma, np.float32)
    ln_beta = np.asarray(ln_beta, np.float32)

    Hm = H.reshape(B * N, D)
    # Wh[r,b,n,h,f]
    Wh = np.empty((R, B, N, Hh, hd), np.float32)
    for r in range(R):
        for h in range(Hh):
            Wh[r, :, :, h, :] = (Hm @ W[r, h]).reshape(B, N, hd)
    es = np.einsum("rbnhf,rhf->rbhn", Wh, a_src)  # [R,B,Hh,N]
    ed = np.einsum("rbnhf,rhf->rbhn", Wh, a_dst)

    t_f = np.exp(0.2 * ed)  # [R,B,Hh,N]  (j-indexed)
    invtau = np.exp(0.8 * ed).astype(np.float32)
    # alpha = u / sum(u) is invariant to any per-(r,b,h) rescale of u =
    # m*max(invtau, g); normalize so whp*invtau (term-1 lhsT) stays in f16
    # range: invtau <= 1 makes |whp1| <= |Wh| * t_max.
    scale_c = (1.0 / np.maximum(invtau.max(axis=3, keepdims=True), 1e-30)).astype(
        np.float32
    )
    invtau = invtau * scale_c
    g_all = (np.exp(-0.8 * es) * scale_c).astype(np.float16)  # (i-indexed)

    # packed W~h = Wh * t plus R*t ones column, per batch: [B, R, 128, NT*132] f16
    whp = np.empty((R, B, NT, 128, Hh, 33), np.float32)
    tj = t_f.transpose(0, 1, 3, 2).reshape(R, B, NT, 128, Hh)
    whp[..., :32] = Wh.reshape(R, B, NT, 128, Hh, hd) * tj[..., None]
    whp[..., 32] = R * tj

    # term-1 lhsT for decomposed heads 0-2: whp scaled by invtau_j per
    # partition (j). agg = (whp*invtau)^T @ m + whp^T @ (m*relu(g-invtau)).
    itj = invtau.transpose(0, 1, 3, 2).reshape(R, B, NT, 128, Hh)
    whp1 = (whp[..., :NDH, :] * itj[..., :NDH, None]).reshape(R, B, NT, 128, W1)
    whp1 = (
        whp1.transpose(1, 0, 3, 2, 4).reshape(B, R, 128, NT * W1).astype(np.float16)
    )
    whp1 = np.ascontiguousarray(whp1)

    whp = (
        whp.reshape(R, B, NT, 128, HW)
        .transpose(1, 0, 3, 2, 4)
        .reshape(B, R, 128, NT * HW)
        .astype(np.float16)
    )
    whp = np.ascontiguousarray(whp)

    # invtau scalar columns [B, R, 128, NT*Hh] f32 (and negated, for ACT bias)
    itc = np.ascontiguousarray(
        invtau.reshape(R, B, Hh, NT, 128).transpose(1, 0, 4, 3, 2)
    ).reshape(B, R, 128, NT * Hh)
    itc = np.ascontiguousarray(itc)
    itn = np.ascontiguousarray(-itc)

    # raw 0/1 mask, transposed: [R,B,j,i_all] fp16
    At = A.transpose(0, 1, 3, 2)
    mq_full = At.astype(np.float16).reshape(R, B, NT, 128, N)

    gmbase = np.stack(
        [
            np.broadcast_to(ln_gamma, (128, D)),
            np.broadcast_to(ln_beta, (128, D)),
        ]
    ).astype(np.float32)
    gmbase = np.ascontiguousarray(gmbase)

    in_maps = []
    for c in range(NCORES):
        b, q = divmod(c, NQ)
        i0 = q * IS
        mq_c = np.ascontiguousarray(
            mq_full[:, b, :, :, i0 : i0 + IS].transpose(0, 2, 1, 3)
        ).reshape(R, 128, NT * IS)
        g_c = g_all[:, b, :, i0 : i0 + IS].reshape(R, Hh * IS)
        g_c = np.ascontiguousarray(np.broadcast_to(g_c[:, None, :], (R, 128, Hh * IS)))
        hres_c = np.ascontiguousarray(H[b, i0 : i0 + IS, :]).reshape(NIT, 128, D)
        in_maps.append(
            {
                "mq": mq_c,
                "whp": whp[b],
                "whp1": whp1[b],
                "gb4": g_c,
                "itc": itc[b],
                "itn": itn[b],
                "hres": hres_c,
                "gmb": gmbase,
            }
        )
    return in_maps


def kernel(H, A, W, a_src, a_dst, ln_gamma, ln_beta):
    from concourse.bass_utils import run_bass_kernel_spmd

    # build-time specialization on the actual LN parameter values (the
    # general path remains available for non-trivial gamma/beta)
    triv = bool(np.all(np.asarray(ln_gamma) == 1.0)) and bool(
        np.all(np.asarray(ln_beta) == 0.0)
    )
    key = ("nc", triv)
    if key not in _CACHE:
        _CACHE[key] = _build_program(trivial_ln=triv)
    nc = _CACHE["nc"] = _CACHE[key]

    in_maps = _host_pack(H, A, W, a_src, a_dst, ln_gamma, ln_beta)
    res = run_bass_kernel_spmd(nc, in_maps, list(range(NCORES)))

    full = np.empty((B, N, D), np.float32)
    for c in range(NCORES):
        b, q = divmod(c, NQ)
        o = np.asarray(res.results[c]["out"], np.float32).reshape(IS, D)
        full[b, q * IS : (q + 1) * IS, :] = o
    return full



# revision 73
# speedup vs baseline: 1.0203x; 1.0203x over previous
"""Multi-relation GAT layer on 8 Trainium2 NeuronCores.

Sharding: cores 0-3 own batch 0, cores 4-7 own batch 1; within a batch each
core owns a quarter of the destination rows (IS=512) for ALL R relations, so
the relation-mean, residual and LayerNorm stay core-local (no collectives).

Host precomputes the dense projections (Wh = H@W, attention dots es/ed) and
all scalar exponentials; the device does the heavy O(R*Hh*N*IS) masked
attention.

Algebra: scores are rank-1 before the leaky-relu (s[i,j] = es_i + ed_j);
softmax over j is invariant to per-i scaling, and per-j scaling folds into
the aggregation weights (linear over j). Dividing exp(lrelu(s)) by exp(es_i)
(per-i) and t_j = exp(0.2*ed_j) (per-j, folded into Wh) leaves

    u[j,i] = m[j,i] * max(invtau_j, g_i)
      invtau = exp(0.8*ed), g = exp(-0.8*es)   (host-precomputed; rescaled
                                               per (r,b,h) so invtau <= 1 --
                                               alpha is scale-invariant)
      W~h = Wh * t,  ones column = R * t       (host-folded)

The max is decomposed as invtau_j + relu(g_i - invtau_j), which splits the
aggregation into a mask-only matmul plus a correction:

    agg^T = (W~h * invtau)^T @ m  +  W~h^T @ (m * relu(g - invtau))

Term 1 (lhsT host-folded, incl. the R*t*invtau denominator column) depends
only on the mask, so the PE runs it while the elementwise engines produce
the correction. The relu units (one per (jt, h), per-partition scalar
invtau) are load-balanced between ACT (Relu activation, bias = -invtau) and
DVE (tensor_scalar sub+max, 2x mode); the mask multiply is a single batched
2x-mode tensor_tensor per j-tile group with the mask broadcast over heads
via a stride-0 middle AP dim.

PSUM: two banks per relation (double-buffered), two heads per bank at
32-aligned partition bases 0/64, so per-head matmul pairs land in distinct
PE column groups and run concurrently (~30ns stagger). Term-1 jt0 carries
start=True; term-2 jt15 carries stop.

Epilogue per (r, i-tile): whole-bank ACT drain to SBUF, one paired PE
transpose per bank to [i, (f,den)] layout, ACT gathers the four denominator
columns, DVE reciprocal, then fused DVE STTs do acc_h = num_h*rec_h +
(residual | acc_h). Pipelined one relation behind the hot loop. Final LN:
negated mean via STT accumulation, variance on ACT (Square, bias=-mu,
accumulated), Sqrt table preloaded at startup.

(GpSimd/Pool is left idle on purpose: tensor ops there measured ~30x slow
and stall the DVE via shared SBUF ports.)
"""

import sys

sys.path.insert(0, "/opt/trn_rl_repo")

import numpy as np

R, B, N, D, Hh, hd = 3, 2, 2048, 128, 4, 32
NCORES = 8
NQ = 4  # i-shards per batch
IS = N // NQ  # 512 dst rows per core
NIT = IS // 128  # 4 partition tiles of dst rows
NT = N // 128  # 16 j tiles
LN_EPS = 1e-5
HW = Hh * 33  # 132 packed Wh cols per j-tile (32 wh + 1 ones per head)
GJT = 4  # max j-tiles per v/u buffer
GROUPS = [(0, 4), (4, 4), (8, 4), (12, 3), (15, 1)]  # (start jt, len) taper
NDH = 4  # all heads use the max = invtau + relu(g - invtau) decomposition
W1 = NDH * 33  # term-1 (mask-matmul) lhsT columns: (wh*invtau | R*t*invtau)
ACT_PER_R = 37  # of NT*Hh=64 relu units per relation: how many on ACT ...
GPS_PER_R = 0  # ... and on GpSimd (tensor ops measured ~30x slow there!)

_CACHE = {}


def _build_program(trivial_ln=False):
    import concourse.bass as bass
    import concourse.mybir as mybir
    import concourse.tile as tile
    from concourse import bacc
    from concourse.masks import make_identity
    from contextlib import ExitStack

    f32 = mybir.dt.float32
    f16 = mybir.dt.float16
    Alu = mybir.AluOpType
    Act = mybir.ActivationFunctionType

    nc = bacc.Bacc("TRN2", target_bir_lowering=False, debug=False)
    mq = nc.declare_dram_parameter("mq", [R, 128, NT * IS], f16, isOutput=False)
    whp = nc.declare_dram_parameter("whp", [R, 128, NT * HW], f16, isOutput=False)
    whp1 = nc.declare_dram_parameter("whp1", [R, 128, NT * W1], f16, isOutput=False)
    gb4 = nc.declare_dram_parameter("gb4", [R, 128, Hh * IS], f16, isOutput=False)
    itc = nc.declare_dram_parameter("itc", [R, 128, NT * Hh], f32, isOutput=False)
    itn = nc.declare_dram_parameter("itn", [R, 128, NT * Hh], f32, isOutput=False)
    hres = nc.declare_dram_parameter("hres", [NIT, 128, D], f32, isOutput=False)
    gmb = nc.declare_dram_parameter("gmb", [2, 128, D], f32, isOutput=False)
    out = nc.declare_dram_parameter("out", [NIT, 128, D], f32, isOutput=True)

    with ExitStack() as ctx:
        tc = ctx.enter_context(tile.TileContext(nc))
        const = ctx.enter_context(tc.tile_pool(name="const", bufs=1))
        mq_pool = ctx.enter_context(tc.tile_pool(name="mq", bufs=2))
        mq0_pool = ctx.enter_context(tc.tile_pool(name="mq0", bufs=3))
        v_pool = ctx.enter_context(tc.tile_pool(name="v", bufs=2))
        u_pool = ctx.enter_context(tc.tile_pool(name="u", bufs=3))
        aggsb_pool = ctx.enter_context(tc.tile_pool(name="aggsb", bufs=3))
        small = ctx.enter_context(tc.tile_pool(name="small", bufs=8))
        epi_pool = ctx.enter_context(tc.tile_pool(name="epi", bufs=6))
        psum_agg = ctx.enter_context(tc.tile_pool(name="pagg", bufs=2, space="PSUM"))
        psum_tp = ctx.enter_context(tc.tile_pool(name="ptp", bufs=4, space="PSUM"))

        # ---- constants / per-relation operands ----
        # DMA priority order: r=0's itc + the h=0 slice of gb4 first (unblocks
        # the first v-units), then the first mask group (unblocks the first
        # TT), then whp[0] (unblocks matmuls), then everything else.
        ident = const.tile([128, 128], f32, tag="ident")

        whp_sb, whp1_sb, gb4_sb, itc_sb, itn_sb = [], [], [], [], []
        mq_pre = {}
        for r in range(R):
            q = const.tile([128, NT * Hh], f32, tag=f"itc{r}")
            itc_sb.append(q)
            q2 = const.tile([128, NT * Hh], f32, tag=f"itn{r}")
            itn_sb.append(q2)
            g = const.tile([128, Hh * IS], f16, tag=f"gb4{r}")
            gb4_sb.append(g)
            w = const.tile([128, NT * HW], f16, tag=f"whp{r}")
            whp_sb.append(w)
            w1 = const.tile([128, NT * W1], f16, tag=f"whp1_{r}")
            whp1_sb.append(w1)

        nc.sync.dma_start(itc_sb[0][:], itc[0])
        nc.sync.dma_start(gb4_sb[0][:], gb4[0])
        # itn = -itc computed on-device: saves early DMA descriptor issues
        nc.vector.tensor_scalar_mul(itn_sb[0][:], itc_sb[0][:], -1.0)
        tiles = []
        for k, (j0, gsz) in enumerate(GROUPS):
            pool_k = mq0_pool if k == 0 else mq_pool
            m = pool_k.tile([128, gsz * IS], f16, tag=f"mqg{k}", name=f"mq0_{k}")
            nc.sync.dma_start(m[:], mq[0][:, j0 * IS : (j0 + gsz) * IS])
            tiles.append(m)
            if k == 0:  # whp right after the lead mask
                nc.sync.dma_start(whp1_sb[0][:], whp1[0])
                nc.sync.dma_start(whp_sb[0][:], whp[0])
        mq_pre[0] = tiles

        make_identity(nc, ident[:])

        hres_sb, acc = [], []
        for t in range(NIT):
            hh = const.tile([128, D], f32, tag=f"hres{t}")
            nc.sync.dma_start(hh[:], hres[t])
            hres_sb.append(hh)
            acc_t = const.tile([128, D], f32, tag=f"acc{t}", name=f"acc{t}")
            acc.append(acc_t)
        gam = const.tile([128, D], f32, tag="gam")
        nc.sync.dma_start(gam[:], gmb[0])
        bet = const.tile([128, D], f32, tag="bet")
        nc.sync.dma_start(bet[:], gmb[1])
        eps_b = const.tile([128, 1], f32, tag="eps_b")
        nc.gpsimd.memset(eps_b[:], LN_EPS)
        # Touch Sqrt once now so the ACT function-table load (1.3us) happens
        # during the DMA-bound prologue instead of inside the LN tail.
        warm = const.tile([128, 1], f32, tag="warm", name="warm")
        nc.scalar.activation(warm[:], eps_b[:], Act.Sqrt, bias=eps_b[:])

        # ---- hot loop over relations ----
        pend = []

        def _epi_units(item):
            er, asbs = item
            for it in range(NIT):
                def quad(it=it):
                    # stage-ordered to avoid DVE<->PE ping-pong: paired
                    # transposes (two heads per bank), dens gather (ACT),
                    # recip, then fused (num*rec)+acc STTs on the DVE
                    tps = []
                    for bk in range(2):
                        tp = psum_tp.tile([128, 97], f32, tag="tp", name="tp")
                        at, _ = asbs[bk * 2]
                        nc.tensor.transpose(
                            tp[:],
                            at[0:97, it * 128 : (it + 1) * 128],
                            ident[0:97, 0:97],
                        )
                        tps.append(tp)
                    dens = small.tile([128, Hh], f32, tag="dens", name="dens")
                    for h in range(Hh):
                        c0 = (h % 2) * 64
                        nc.scalar.copy(
                            dens[:, h : h + 1], tps[h // 2][:, c0 + 32 : c0 + 33]
                        )
                    recs = small.tile([128, Hh], f32, tag="recs", name="recs")
                    nc.vector.reciprocal(recs[:], dens[:])
                    for h in range(Hh):
                        c0 = (h % 2) * 64
                        dst = acc[it][:, h * hd : (h + 1) * hd]
                        # acc_h = num_h * rec_h + (residual | acc_h)
                        nc.vector.scalar_tensor_tensor(
                            out=dst,
                            in0=tps[h // 2][:, c0 : c0 + 32],
                            scalar=recs[:, h : h + 1],
                            in1=hres_sb[it][:, h * hd : (h + 1) * hd]
                            if er == 0
                            else dst,
                            op0=Alu.mult,
                            op1=Alu.add,
                        )
                yield quad

        def _emit_epi(item):
            for u_ in _epi_units(item):
                u_()

        # Engine schedule for the relu units. First group (jt<4) stays
        # DVE-heavy: at relation boundaries the ACT queue is still draining
        # the previous epilogue, and the group-0 TT would stall on it.
        NU = NT * Hh
        G0U = GROUPS[0][1] * Hh  # units in the first group
        act_g0 = 4
        act_rest = ACT_PER_R - act_g0
        pattern = []
        accb = 0.0
        for ui in range(NU):
            if ui < G0U:
                pattern.append("A" if ui % Hh == 1 and ui // Hh < act_g0 else "D")
            else:
                accb += act_rest / (NU - G0U)
                if accb >= 1.0:
                    accb -= 1.0
                    pattern.append("A")
                else:
                    pattern.append("D")

        for r in range(R):
            if r + 1 < R:
                # defer the next relation's constants: keeps the early DMA
                # window clear for relation 0's masks
                nc.sync.dma_start(itc_sb[r + 1][:], itc[r + 1])
                nc.vector.tensor_scalar_mul(itn_sb[r + 1][:], itc_sb[r + 1][:], -1.0)
                nc.sync.dma_start(gb4_sb[r + 1][:], gb4[r + 1])
                nc.sync.dma_start(whp1_sb[r + 1][:], whp1[r + 1])
                nc.sync.dma_start(whp_sb[r + 1][:], whp[r + 1])
            if r in mq_pre:
                m_tiles = mq_pre[r]
            else:
                m_tiles = []
                for k, (j0, gsz) in enumerate(GROUPS):
                    pool_k = mq0_pool if k == 0 else mq_pool
                    m = pool_k.tile(
                        [128, gsz * IS], f16, tag=f"mqg{k}", name=f"mq{r}_{k}"
                    )
                    nc.sync.dma_start(m[:], mq[r][:, j0 * IS : (j0 + gsz) * IS])
                    m_tiles.append(m)

            # Two PSUM banks, two heads each at 32-aligned bases (0 and 64):
            # pairs of matmuls hit distinct PE column groups and run
            # concurrently (~4ns stagger).
            aggA = psum_agg.tile([97, IS], f32, tag="aggA", name=f"aggA{r}")
            aggB = psum_agg.tile([97, IS], f32, tag="aggB", name=f"aggB{r}")
            aggp = [aggA[0:33, :], aggA[64:97, :], aggB[0:33, :], aggB[64:97, :]]
            epi_iter = iter(_epi_units(pend.pop(0))) if pend else None

            for g, (j0, gsz) in enumerate(GROUPS):
                # term-1: (whp*invtau)^T @ mask — depends only on the mask
                # DMA, so the PE can run these while DVE/ACT produce p
                for jl in range(gsz):
                    jt = j0 + jl
                    for h in range(Hh):
                        nc.tensor.matmul(
                            aggp[h],
                            lhsT=whp1_sb[r][:, jt * W1 + h * 33 : jt * W1 + (h + 1) * 33],
                            rhs=m_tiles[g][:, jl * IS : (jl + 1) * IS],
                            start=(jt == 0),
                            stop=False,
                        )
                v = v_pool.tile([128, GJT * Hh * IS], f16, tag="v")
                for jl in range(gsz):
                    jt = j0 + jl
                    for h in range(Hh):
                        vsl = v[:, (jl * Hh + h) * IS : (jl * Hh + h + 1) * IS]
                        gsl = gb4_sb[r][:, h * IS : (h + 1) * IS]
                        # p-pass input: relu(g_i - invtau_j)
                        eng = pattern[jt * Hh + h]
                        if eng == "A":
                            nc.scalar.activation(
                                vsl,
                                gsl,
                                Act.Relu,
                                bias=itn_sb[r][:, jt * Hh + h : jt * Hh + h + 1],
                            )
                        else:
                            veng = nc.vector if eng == "D" else nc.gpsimd
                            veng.tensor_scalar(
                                out=vsl,
                                in0=gsl,
                                scalar1=itc_sb[r][:, jt * Hh + h : jt * Hh + h + 1],
                                scalar2=0.0,
                                op0=Alu.subtract,
                                op1=Alu.max,
                            )
                u = u_pool.tile([128, GJT * Hh * IS], f16, tag="u")
                m3 = m_tiles[g][:].rearrange("p (a i) -> p a i", a=gsz)
                nc.vector.tensor_mul(
                    u[:, : gsz * Hh * IS].rearrange("p (a h i) -> p a h i", a=gsz, h=Hh),
                    v[:, : gsz * Hh * IS].rearrange("p (a h i) -> p a h i", a=gsz, h=Hh),
                    m3[:, :, None, :].broadcast_to([128, gsz, Hh, IS]),
                )
                for jl in range(gsz):
                    jt = j0 + jl
                    for h in range(Hh):
                        nc.tensor.matmul(
                            aggp[h],
                            lhsT=whp_sb[r][:, jt * HW + h * 33 : jt * HW + (h + 1) * 33],
                            rhs=u[:, (jl * Hh + h) * IS : (jl * Hh + h + 1) * IS],
                            start=False,
                            stop=(jt == NT - 1),
                        )
                if epi_iter is not None:
                    u_ = next(epi_iter, None)  # one it-quad per group
                    if u_ is not None:
                        u_()

            # ---- drain PSUM quickly (frees agg banks for next r's chains).
            # Whole-bank copies keep src/dst partitions aligned (elementwise
            # engines cannot shift partitions). Last relation: drain is on
            # the critical tail with the DVE idle, so split it ACT/DVE. ----
            asbA = aggsb_pool.tile([97, IS], f32, tag="aggsbA")
            asbB = aggsb_pool.tile([97, IS], f32, tag="aggsbB")
            nc.scalar.copy(asbA[:], aggA[:])
            if r == R - 1:
                nc.vector.tensor_copy(asbB[:], aggB[:])
            else:
                nc.scalar.copy(asbB[:], aggB[:])
            asbs = [(asbA, 0), (asbA, 64), (asbB, 0), (asbB, 64)]
            pend.append((r, asbs))

        # last relation: 4-wide batched per dst partition tile
        er, asbs = pend.pop(0)
        xs, mus, xcs, stds = [], [], [], []
        contribs = [
            epi_pool.tile([128, D], f32, tag="fincon", name=f"contribf{it}")
            for it in range(NIT)
        ]
        for it in range(NIT):
            tps = []
            for bk in range(2):
                tp = psum_tp.tile([128, 97], f32, tag="tp", name="tpf")
                at, _ = asbs[bk * 2]
                nc.tensor.transpose(
                    tp[:], at[0:97, it * 128 : (it + 1) * 128], ident[0:97, 0:97]
                )
                tps.append(tp)
            # all-DVE here: the DVE is idle in the tail while ACT round
            # trips would serialize the chain
            recsf = small.tile([128, Hh], f32, tag="recsf", name="recsf")
            for h in range(Hh):
                c0 = (h % 2) * 64
                nc.vector.reciprocal(
                    recsf[:, h : h + 1], tps[h // 2][:, c0 + 32 : c0 + 33]
                )
            for h in range(Hh):
                c0 = (h % 2) * 64
                nc.vector.tensor_scalar(
                    out=contribs[it][:, h * hd : (h + 1) * hd],
                    in0=tps[h // 2][:, c0 : c0 + 32],
                    scalar1=recsf[:, h : h + 1],
                    scalar2=None,
                    op0=Alu.mult,
                )
            # LN stages: x + (negated) mean on DVE; variance on ACT (Square
            # with bias=-mu, accumulated) so the two run concurrently
            x = epi_pool.tile([128, D], f32, tag="x", name=f"x{it}")
            musum = small.tile([128, 1], f32, tag="mu", name=f"mu{it}")
            nc.vector.scalar_tensor_tensor(
                out=x[:], in0=acc[it][:], scalar=0.0, in1=contribs[it][:],
                op0=Alu.add, op1=Alu.add, accum_out=musum[:],
            )
            nc.vector.tensor_scalar_mul(musum[:], musum[:], -1.0 / D)  # = -mu
            xc = epi_pool.tile([128, D], f32, tag="xc", name=f"xc{it}")
            nc.vector.tensor_scalar(
                out=xc[:], in0=x[:], scalar1=musum[:], scalar2=None,
                op0=Alu.add,
            )
            sq = epi_pool.tile([128, D], f32, tag="sq", name=f"sq{it}")
            vsum = small.tile([128, 1], f32, tag="vs", name=f"vs{it}")
            nc.scalar.activation(
                sq[:], x[:], Act.Square, bias=musum[:], accum_out=vsum[:]
            )
            std = small.tile([128, 1], f32, tag="std", name=f"std{it}")
            nc.scalar.activation(std[:], vsum[:], Act.Sqrt, bias=eps_b[:], scale=1.0 / D)
            xs.append(x)
            mus.append(musum)
            xcs.append(xc)
            stds.append(std)

        # ---- epilogue: LayerNorm finish (stage-batched) ----
        xcgs, rstds = [], []
        if not trivial_ln:
            for t in range(NIT):
                # fills the DVE while ACT does the sqrts (rstd-independent)
                xcg = epi_pool.tile([128, D], f32, tag="xg", name=f"xcg{t}")
                nc.vector.tensor_mul(xcg[:], xcs[t][:], gam[:])
                xcgs.append(xcg)
        for t in range(NIT):
            rstd = small.tile([128, 1], f32, tag="rstd", name=f"rstd{t}")
            nc.vector.reciprocal(rstd[:], stds[t][:])
            rstds.append(rstd)
        for t in range(NIT):
            xo = epi_pool.tile([128, D], f32, tag="xo", name=f"xo{t}")
            if trivial_ln:
                # gamma==1, beta==0 (checked at build time): xo = xc*rstd
                nc.vector.tensor_scalar(
                    out=xo[:], in0=xcs[t][:], scalar1=rstds[t][:],
                    scalar2=None, op0=Alu.mult,
                )
            else:
                # xo = (xc*gamma)*rstd + beta in one fused op
                nc.vector.scalar_tensor_tensor(
                    out=xo[:], in0=xcgs[t][:], scalar=rstds[t][:], in1=bet[:],
                    op0=Alu.mult, op1=Alu.add,
                )
            nc.sync.dma_start(out[t], xo[:])

    nc.compile()
    return nc


def _host_pack(H, A, W, a_src, a_dst, ln_gamma, ln_beta):
    H = np.asarray(H, np.float32)
    A = np.asarray(A)
    W = np.asarray(W, np.float32)
    a_src = np.asarray(a_src, np.float32)
    a_dst = np.asarray(a_dst, np.float32)
    ln_gamma = np.asarray(ln_gamma, np.float32)
    ln_beta = np.asarray(ln_beta, np.float32)

    Hm = H.reshape(B * N, D)
    # Wh[r,b,n,h,f]
    Wh = np.empty((R, B, N, Hh, hd), np.float32)
    for r in range(R):
        for h in range(Hh):
            Wh[r, :, :, h, :] = (Hm @ W[r, h]).reshape(B, N, hd)
    es = np.einsum("rbnhf,rhf->rbhn", Wh, a_src)  # [R,B,Hh,N]
    ed = np.einsum("rbnhf,rhf->rbhn", Wh, a_dst)

    t_f = np.exp(0.2 * ed)  # [R,B,Hh,N]  (j-indexed)
    invtau = np.exp(0.8 * ed).astype(np.float32)
    # alpha = u / sum(u) is invariant to any per-(r,b,h) rescale of u =
    # m*max(invtau, g); normalize so whp*invtau (term-1 lhsT) stays in f16
    # range: invtau <= 1 makes |whp1| <= |Wh| * t_max.
    scale_c = (1.0 / np.maximum(invtau.max(axis=3, keepdims=True), 1e-30)).astype(
        np.float32
    )
    invtau = invtau * scale_c
    g_all = (np.exp(-0.8 * es) * scale_c).astype(np.float16)  # (i-indexed)

    # packed W~h = Wh * t plus R*t ones column, per batch: [B, R, 128, NT*132] f16
    whp = np.empty((R, B, NT, 128, Hh, 33), np.float32)
    tj = t_f.transpose(0, 1, 3, 2).reshape(R, B, NT, 128, Hh)
    whp[..., :32] = Wh.reshape(R, B, NT, 128, Hh, hd) * tj[..., None]
    whp[..., 32] = R * tj

    # term-1 lhsT for decomposed heads 0-2: whp scaled by invtau_j per
    # partition (j). agg = (whp*invtau)^T @ m + whp^T @ (m*relu(g-invtau)).
    itj = invtau.transpose(0, 1, 3, 2).reshape(R, B, NT, 128, Hh)
    whp1 = (whp[..., :NDH, :] * itj[..., :NDH, None]).reshape(R, B, NT, 128, W1)
    whp1 = (
        whp1.transpose(1, 0, 3, 2, 4).reshape(B, R, 128, NT * W1).astype(np.float16)
    )
    whp1 = np.ascontiguousarray(whp1)

    whp = (
        whp.reshape(R, B, NT, 128, HW)
        .transpose(1, 0, 3, 2, 4)
        .reshape(B, R, 128, NT * HW)
        .astype(np.float16)
    )
    whp = np.ascontiguousarray(whp)

    # invtau scalar columns [B, R, 128, NT*Hh] f32 (and negated, for ACT bias)
    itc = np.ascontiguousarray(
        invtau.reshape(R, B, Hh, NT, 128).transpose(1, 0, 4, 3, 2)
    ).reshape(B, R, 128, NT * Hh)
    itc = np.ascontiguousarray(itc)
    itn = np.ascontiguousarray(-itc)

    # raw 0/1 mask, transposed: [R,B,j,i_all] fp16
    At = A.transpose(0, 1, 3, 2)
    mq_full = At.astype(np.float16).reshape(R, B, NT, 128, N)

    gmbase = np.stack(
        [
            np.broadcast_to(ln_gamma, (128, D)),
            np.broadcast_to(ln_beta, (128, D)),
        ]
    ).astype(np.float32)
    gmbase = np.ascontiguousarray(gmbase)

    in_maps = []
    for c in range(NCORES):
        b, q = divmod(c, NQ)
        i0 = q * IS
        mq_c = np.ascontiguousarray(
            mq_full[:, b, :, :, i0 : i0 + IS].transpose(0, 2, 1, 3)
        ).reshape(R, 128, NT * IS)
        g_c = g_all[:, b, :, i0 : i0 + IS].reshape(R, Hh * IS)
        g_c = np.ascontiguousarray(np.broadcast_to(g_c[:, None, :], (R, 128, Hh * IS)))
        hres_c = np.ascontiguousarray(H[b, i0 : i0 + IS, :]).reshape(NIT, 128, D)
        in_maps.append(
            {
                "mq": mq_c,
                "whp": whp[b],
                "whp1": whp1[b],
                "gb4": g_c,
                "itc": itc[b],
                "itn": itn[b],
                "hres": hres_c,
                "gmb": gmbase,
            }
        )
    return in_maps


def kernel(H, A, W, a_src, a_dst, ln_gamma, ln_beta):
    from concourse.bass_utils import run_bass_kernel_spmd

    # build-time specialization on the actual LN parameter values (the
    # general path remains available for non-trivial gamma/beta)
    triv = bool(np.all(np.asarray(ln_gamma) == 1.0)) and bool(
        np.all(np.asarray(ln_beta) == 0.0)
    )
    key = ("nc", triv)
    if key not in _CACHE:
        _CACHE[key] = _build_program(trivial_ln=triv)
    nc = _CACHE["nc"] = _CACHE[key]

    in_maps = _host_pack(H, A, W, a_src, a_dst, ln_gamma, ln_beta)
    res = run_bass_kernel_spmd(nc, in_maps, list(range(NCORES)))

    full = np.empty((B, N, D), np.float32)
    for c in range(NCORES):
        b, q = divmod(c, NQ)
        o = np.asarray(res.results[c]["out"], np.float32).reshape(IS, D)
        full[b, q * IS : (q + 1) * IS, :] = o
    return full

